# revision 12
# baseline (speedup 1.0000x reference)
"""Trainium2 Bass kernel for nn_ChunkedRichPoolMLP (segment_reduce).

Strategy (8 NeuronCores, SPMD):
  - Token-shard: core i owns tokens [128*i, 128*(i+1)) of T=1024. All chunk
    boundaries (seg 16/32/64) align to the 128-token shard, so every core
    computes the features for its own 14 local chunks (8+4+2) x 12 feature
    tiles of D=128 -> 21504 feature rows.
  - This row-shards W1: core i streams only its 21504x512 slice (bf16).
  - Each core computes a partial h = flat_local @ W1_local (PSUM accum over
    168 k-tiles), AllReduce([64,512]) across cores, then every core runs the
    tiny MLP tail (gelu, W2, clip) redundantly; core 0's output is returned.

  Feature computation (per core), layouts:
  - B-layout [t=128 partitions, (b=64, d=128) free] for segment sums: per-b
    stationary matmul lhsT=Z_b [t,d], rhs=[G_mean | exp-weights*mask] gives
    mean.T and (unnormalized) attn.T in [d, c] feature layout directly.
  - E[x^2] via ACT Square + same per-b matmul against G_mean.
  - A-layout [d=128 partitions, (b, t) free] (host-marshalled for Z,
    PE-transposed for dZ/ddZ) for the max feature via shifted tensor-max
    trees on the free dim.
  - Scores: u = Z_b^T q (per-b stationary matmul on A-layout), then first/
    second difference matrices (host constants) applied on the t-partition
    axis via tiny matmuls, exp on ACT (no stabilizer needed: |s|<~8), then
    softmax normalization folded in AFTER the attn matmul via a ones-matmul
    denominator replication + reciprocal multiply.
  - dZ/ddZ in B-layout from shifted DMA loads of the same (haloed) DRAM
    tensor + DVE subtracts. Core-0 halo is host-padded with Z[0] replicated,
    which makes dZ[0]=0, ddZ[0]=0, ddZ[1]=Z[1]-Z[0] exact with the uniform
    formulas (no per-core branching).
"""

import math
import os
import sys

sys.path.insert(0, "/opt/trn_rl_repo")

import numpy as np
import ml_dtypes

B, T, D = 64, 1024, 128
TPS = 16
SPLITS = (1, 2, 4)
HIDDEN, OUT = 512, 3
NCORES = 8
TL = T // NCORES          # 128 tokens per core
NCH = 14                  # local chunks: 8 + 4 + 2
FEAT_ROWS = NCH * 12 * D  # 21504
KT = FEAT_ROWS // 128     # 168 k-tiles
# local split table: (c0, n_chunks, seg)
SPLIT_TAB = [(0, 8, 16), (8, 4, 32), (12, 2, 64)]

BF16 = ml_dtypes.bfloat16

_CACHE = {}


def _build_bass():
    import concourse.bass as bass
    import concourse.tile as tile
    import concourse.mybir as mybir
    from concourse import bacc
    from concourse.bass_types import AP  # noqa

    fp32 = mybir.dt.float32
    bf16 = mybir.dt.bfloat16
    ALU = mybir.AluOpType
    ACTF = mybir.ActivationFunctionType

    nc = bacc.Bacc("TRN2", target_bir_lowering=False, debug=False,
                   num_devices=NCORES)

    # ---- I/O ----
    d_zb = nc.declare_dram_parameter("zb", [TL + 2, B, D], bf16, isOutput=False)
    d_za = nc.declare_dram_parameter("za", [D, B, TL + 2], bf16, isOutput=False)
    d_w1 = nc.declare_dram_parameter("w1", [KT, 128, HIDDEN], bf16, isOutput=False)
    d_q3 = nc.declare_dram_parameter("q3", [D, 3], bf16, isOutput=False)
    d_gm = nc.declare_dram_parameter("gm", [TL, NCH], bf16, isOutput=False)
    d_gm01 = nc.declare_dram_parameter("gm01", [TL, NCH], bf16, isOutput=False)
    d_idn = nc.declare_dram_parameter("idn", [128, 128], bf16, isOutput=False)
    d_ones = nc.declare_dram_parameter("ones", [128, 128], bf16, isOutput=False)
    d_sd1 = nc.declare_dram_parameter("sd1", [TL, TL], fp32, isOutput=False)
    d_sd2 = nc.declare_dram_parameter("sd2", [TL, TL], fp32, isOutput=False)
    d_sh1 = nc.declare_dram_parameter("sh1", [2, TL], fp32, isOutput=False)
    d_sh2 = nc.declare_dram_parameter("sh2", [2, TL], fp32, isOutput=False)
    d_w2 = nc.declare_dram_parameter("w2", [4, 128, OUT], bf16, isOutput=False)
    d_b1 = nc.declare_dram_parameter("b1s", [128, 4], fp32, isOutput=False)
    d_b2 = nc.declare_dram_parameter("b2s", [OUT, 1], fp32, isOutput=False)
    d_y = nc.declare_dram_parameter("y", [B, OUT], fp32, isOutput=True)

    # collective bounce buffers (must be Internal DRAM)
    d_hpart = nc.dram_tensor("hpart", [B, HIDDEN], fp32)
    d_hred = nc.dram_tensor("hred", [B, HIDDEN], fp32, addr_space="Shared")

    from contextlib import ExitStack
    with tile.TileContext(nc) as tc, ExitStack() as ctx:
        consts = ctx.enter_context(tc.tile_pool(name="consts", bufs=1))
        big = ctx.enter_context(tc.tile_pool(name="big", bufs=1))
        w1p = ctx.enter_context(tc.tile_pool(name="w1p", bufs=6))
        sqp = ctx.enter_context(tc.tile_pool(name="sqp", bufs=2))
        mxp = ctx.enter_context(tc.tile_pool(name="mxp", bufs=2))
        smal = ctx.enter_context(tc.tile_pool(name="smal", bufs=2))
        s16k = ctx.enter_context(tc.tile_pool(name="s16k", bufs=2))
        psp = ctx.enter_context(tc.tile_pool(name="psp", bufs=1, space="PSUM"))

        dma = nc.sync.dma_start
        sdma = nc.gpsimd.dma_start

        # ---- load constants / inputs ----
        za_sb = big.tile([D, B, TL + 2], bf16)        # [d, (b, t_ext)]
        zb_sb = big.tile([TL, B * D], bf16)
        zs1_sb = s16k.tile([TL, B * D], bf16, tag="s16k")
        zs2_sb = s16k.tile([TL, B * D], bf16, tag="s16k")
        sdma(za_sb[:, :, :], d_za.ap())
        zbap = d_zb.ap()                              # [130, B, D]
        sdma(zb_sb[:, :], zbap[2:TL + 2].rearrange("t b d -> t (b d)"))
        sdma(zs1_sb[:, :], zbap[1:TL + 1].rearrange("t b d -> t (b d)"))
        sdma(zs2_sb[:, :], zbap[0:TL].rearrange("t b d -> t (b d)"))

        q3_sb = consts.tile([D, 3], bf16)
        gm_sb = consts.tile([TL, NCH], bf16)
        gm01_sb = consts.tile([TL, NCH], bf16)
        idn_sb = consts.tile([128, 128], bf16)
        ones_sb = consts.tile([128, 128], bf16)
        sd1_sb = consts.tile([TL, TL], fp32)
        sd2_sb = consts.tile([TL, TL], fp32)
        sh1_sb = consts.tile([2, TL], fp32)
        sh2_sb = consts.tile([2, TL], fp32)
        w2_sb = consts.tile([128, 4, OUT], bf16)
        b1_sb = consts.tile([128, 4], fp32)
        b2_sb = consts.tile([OUT, 1], fp32)
        for t_, d_ in ((q3_sb, d_q3), (gm_sb, d_gm), (gm01_sb, d_gm01),
                       (idn_sb, d_idn), (ones_sb, d_ones), (sd1_sb, d_sd1),
                       (sd2_sb, d_sd2), (sh1_sb, d_sh1), (sh2_sb, d_sh2),
                       (b1_sb, d_b1), (b2_sb, d_b2)):
            sdma(t_[(slice(None),) * len(t_.shape)], d_.ap())
        sdma(w2_sb[:, :, :], d_w2.ap().rearrange("k p o -> p k o"))

        # gall[sig]: [t, (b, 28)]; cols 0:14 = G_mean (1/seg), 14:28 = e*mask01
        gall = [big.tile([TL, B, 2 * NCH], bf16, name=f"gall{s}")
                for s in range(3)]
        gm_bcast = bass.AP(tensor=d_gm.ap().tensor, offset=0,
                           ap=[[NCH, TL], [0, B], [1, NCH]])
        for s in range(3):
            sdma(gall[s][:, :, 0:NCH], gm_bcast)

        # ---- deltas in B-layout ----
        dzb_sb = big.tile([TL, B * D], bf16)
        ddzb_sb = big.tile([TL, B * D], bf16)
        nc.vector.tensor_sub(dzb_sb[:, :], zb_sb[:, :], zs1_sb[:, :])
        # ddz = zb - 2*zs1 + zs2
        nc.vector.scalar_tensor_tensor(ddzb_sb[:, :], zs1_sb[:, :], -2.0,
                                       zb_sb[:, :], op0=ALU.mult, op1=ALU.add)
        nc.vector.tensor_add(ddzb_sb[:, :], ddzb_sb[:, :], zs2_sb[:, :])

        # ---- scores -> e ----
        # u[t, (b,3)] via per-b stationary matmuls on A-layout
        u_ps = psp.tile([TL, B * 3], fp32, tag="sps", bufs=2)
        for b in range(B):
            nc.tensor.matmul(u_ps[:, 3 * b:3 * b + 3],
                             lhsT=za_sb[:, b, 2:TL + 2], rhs=q3_sb[:, :])
        u_sb = smal.tile([TL, B * 3], fp32, tag="u")
        nc.vector.tensor_copy(u_sb[:, :], u_ps[:, :])

        # halo u: gather halo cols to a contiguous [d, (j, b)] tile first
        zah_sb = smal.tile([D, 2, B], bf16, tag="zah")
        nc.vector.tensor_copy(zah_sb[:, :, :],
                              za_sb[:, :, 0:2].transpose([0, 2, 1]))
        uh_ps = psp.tile([128, 3], fp32, tag="sps", bufs=2)
        nc.tensor.matmul(uh_ps[:, :],
                         lhsT=zah_sb[:, :, :].rearrange("d j b -> d (j b)"),
                         rhs=q3_sb[:, :])
        uh_sb = smal.tile([128, 3], fp32, tag="uh")
        nc.vector.tensor_copy(uh_sb[:, :], uh_ps[:, :])
        # rearrange [(b,j), 3] -> [j=2, (b,3)] with two small DMAs
        uh2_sb = smal.tile([2, B * 3], fp32, tag="uh2")
        for j in range(2):
            dma(uh2_sb[j:j + 1, :].rearrange("j (b s) -> j b s", s=3),
                uh_sb[B * j:B * (j + 1), :])

        u3 = u_sb[:, :].rearrange("t (b s) -> t b s", s=3)
        uh3 = uh2_sb[:, :].rearrange("j (b s) -> j b s", s=3)
        sdz_ps = psp.tile([TL, B], fp32, tag="sps", bufs=2)
        sddz_ps = psp.tile([TL, B], fp32, tag="sps", bufs=2)
        nc.tensor.matmul(sdz_ps[:, :], lhsT=sd1_sb[:, :], rhs=u3[:, :, 1],
                         start=True, stop=False)
        nc.tensor.matmul(sdz_ps[:, :], lhsT=sh1_sb[:, :], rhs=uh3[:, :, 1],
                         start=False, stop=True)
        nc.tensor.matmul(sddz_ps[:, :], lhsT=sd2_sb[:, :], rhs=u3[:, :, 2],
                         start=True, stop=False)
        nc.tensor.matmul(sddz_ps[:, :], lhsT=sh2_sb[:, :], rhs=uh3[:, :, 2],
                         start=False, stop=True)

        e_sb = smal.tile([TL, 3, B], bf16, tag="e")
        nc.scalar.activation(e_sb[:, 0, :], u3[:, :, 0], ACTF.Exp)
        nc.scalar.activation(e_sb[:, 1, :], sdz_ps[:, :], ACTF.Exp)
        nc.scalar.activation(e_sb[:, 2, :], sddz_ps[:, :], ACTF.Exp)

        # unnormalized exp-weighted masks into gall cols 14:28
        for s in range(3):
            for (c0, nch, seg) in SPLIT_TAB:
                nc.vector.tensor_mul(
                    gall[s][:, :, NCH + c0:NCH + c0 + nch],
                    gm01_sb[:, c0:c0 + nch].unsqueeze(1).broadcast_to(
                        (TL, B, nch)),
                    e_sb[:, s, :].unsqueeze(2).broadcast_to((TL, B, nch)))

        # ---- per-signal feature passes + main matmul ----
        h_ps = psp.tile([B, HIDDEN], fp32, tag="h", bufs=1)
        kt = 0
        w1ap = d_w1.ap()
        srcs = [zb_sb, dzb_sb, ddzb_sb]

        for sig in range(3):
            srcB = srcs[sig]
            srcB3 = srcB[:, :].rearrange("t (b d) -> t b d", d=D)
            f_mean = big.tile([128, NCH, B], bf16, tag=f"fmean{sig}")
            f_std = big.tile([128, NCH, B], bf16, tag=f"fstd{sig}")
            f_max = big.tile([128, NCH, B], bf16, tag=f"fmax{sig}")
            f_attn = big.tile([128, NCH, B], bf16, tag=f"fattn{sig}")

            # A-layout source for the max feature
            if sig == 0:
                amax3 = za_sb[:, :, 2:TL + 2]       # [d, b, t]
            else:
                dza = s16k.tile([D, B, TL], bf16, tag="s16k", name=f"dza{sig}")
                for bg in range(16):
                    tp_ps = psp.tile([128, 4, 128], bf16, tag="tps", bufs=2)
                    for k in range(4):
                        b = 4 * bg + k
                        nc.tensor.transpose(tp_ps[:, k, :], srcB3[:, b, :],
                                            idn_sb[:, :])
                    nc.vector.tensor_copy(dza[:, 4 * bg:4 * bg + 4, :],
                                          tp_ps[:, :, :])
                amax3 = dza[:, :, :]

            for bg in range(4):     # 16-b groups
                b0 = 16 * bg
                sq_sb = sqp.tile([TL, 16, D], bf16, tag="sq")
                nc.scalar.activation(sq_sb[:, :, :], srcB3[:, b0:b0 + 16, :],
                                     ACTF.Square)
                ps1 = psp.tile([128, 16, 2 * NCH], fp32, tag="pstat", bufs=3)
                ps2 = psp.tile([128, 16, NCH], fp32, tag="pstat", bufs=3)
                for k in range(16):
                    b = b0 + k
                    nc.tensor.matmul(ps1[:, k, :], lhsT=srcB3[:, b, :],
                                     rhs=gall[sig][:, b, :])
                    nc.tensor.matmul(ps2[:, k, :], lhsT=sq_sb[:, k, :],
                                     rhs=gm_sb[:, :])
                # evac mean / attn (psum is [d, b, c]; flat is [d, c, b])
                nc.vector.tensor_copy(f_mean[:, :, b0:b0 + 16],
                                      ps1[:, :, 0:NCH].transpose([0, 2, 1]))
                nc.vector.tensor_copy(f_attn[:, :, b0:b0 + 16],
                                      ps1[:, :, NCH:2 * NCH].transpose([0, 2, 1]))
                # std = sqrt(max(E[x^2] - mean^2, 0))
                msq = sqp.tile([128, 16, NCH], fp32, tag="msq")
                nc.scalar.activation(
                    msq[:, :, :],
                    f_mean[:, :, b0:b0 + 16].transpose([0, 2, 1]), ACTF.Square)
                var = sqp.tile([128, 16, NCH], fp32, tag="var")
                nc.vector.tensor_sub(var[:, :, :], ps2[:, :, :], msq[:, :, :])
                nc.vector.tensor_scalar_max(var[:, :, :], var[:, :, :], 0.0)
                nc.scalar.activation(f_std[:, :, b0:b0 + 16].transpose([0, 2, 1]),
                                     var[:, :, :], ACTF.Sqrt)
                # max tree over A-layout
                am = amax3[:, b0:b0 + 16, :]
                am8 = am.rearrange("d b (c s) -> d b c s", s=16)
                mx8 = mxp.tile([128, 16, 8, 8], bf16, tag="mx8")
                nc.vector.tensor_max(mx8[:, :, :, :], am8[:, :, :, 0:8],
                                     am8[:, :, :, 8:16])
                mx4 = mxp.tile([128, 16, 8, 4], bf16, tag="mx4")
                nc.vector.tensor_max(mx4[:, :, :, :], mx8[:, :, :, 0:4],
                                     mx8[:, :, :, 4:8])
                mx2 = mxp.tile([128, 16, 8, 2], bf16, tag="mx2")
                nc.vector.tensor_max(mx2[:, :, :, :], mx4[:, :, :, 0:2],
                                     mx4[:, :, :, 2:4])
                f16v = f_max[:, 0:8, b0:b0 + 16].transpose([0, 2, 1])
                nc.vector.tensor_max(f16v, mx2[:, :, :, 0], mx2[:, :, :, 1])
                f16s = f_max[:, :, b0:b0 + 16].rearrange("d c b -> d b c")
                nc.vector.tensor_max(
                    f16s[:, :, 8:12],
                    f16s[:, :, 0:8].rearrange("d b (c k) -> d b c k", k=2)[:, :, :, 0],
                    f16s[:, :, 0:8].rearrange("d b (c k) -> d b c k", k=2)[:, :, :, 1])
                nc.vector.tensor_max(
                    f16s[:, :, 12:14],
                    f16s[:, :, 8:12].rearrange("d b (c k) -> d b c k", k=2)[:, :, :, 0],
                    f16s[:, :, 8:12].rearrange("d b (c k) -> d b c k", k=2)[:, :, :, 1])

            # attn normalization: denom replicated via ones-matmul
            rd_sb = smal.tile([128, B, 2 * NCH], fp32, tag="rd")
            gflat = gall[sig][:, :, :].rearrange("t b c -> t (b c)")
            rdflat = rd_sb[:, :, :].rearrange("t b c -> t (b c)")
            for q in range(4):
                dn_ps = psp.tile([128, 448], fp32, tag="tps", bufs=2)
                nc.tensor.matmul(dn_ps[:, :], lhsT=ones_sb[:, :],
                                 rhs=gflat[:, 448 * q:448 * (q + 1)])
                nc.vector.reciprocal(rdflat[:, 448 * q:448 * (q + 1)],
                                     dn_ps[:, :])
            nc.vector.tensor_mul(f_attn[:, :, :], f_attn[:, :, :],
                                 rd_sb[:, :, NCH:2 * NCH].transpose([0, 2, 1]))

            # main matmul k-tiles for this signal (order: mean,std,max,attn)
            for f_t in (f_mean, f_std, f_max, f_attn):
                for c in range(NCH):
                    wt = w1p.tile([128, HIDDEN], bf16, tag="w1t")
                    dma(wt[:, :], w1ap[kt])
                    nc.tensor.matmul(h_ps[:, :], lhsT=f_t[:, c, :], rhs=wt[:, :],
                                     start=(kt == 0), stop=(kt == KT - 1))
                    kt += 1

        # ---- tail: AllReduce + MLP head ----
        h_sb = smal.tile([B, HIDDEN], fp32, tag="hsb")
        nc.vector.tensor_copy(h_sb[:, :], h_ps[:, :])
        dma(d_hpart.ap(), h_sb[:, :])
        nc.gpsimd.collective_compute(
            "AllReduce", ALU.add, replica_groups=[list(range(NCORES))],
            ins=[d_hpart.ap().opt()], outs=[d_hred.ap().opt()])
        hred_sb = smal.tile([B, HIDDEN], fp32, tag="hred")
        dma(hred_sb[:, :], d_hred.ap())
        hb_sb = smal.tile([B, HIDDEN], bf16, tag="hb")
        nc.vector.tensor_copy(hb_sb[:, :], hred_sb[:, :])
        ht_ps = psp.tile([128, 4, B], bf16, tag="tps", bufs=2)
        for jt in range(4):
            nc.tensor.transpose(ht_ps[:, jt, :], hb_sb[:, 128 * jt:128 * (jt + 1)],
                                idn_sb[0:B, 0:B])
        g_sb = smal.tile([128, 4, B], bf16, tag="g")
        for jt in range(4):
            nc.scalar.activation(g_sb[:, jt, :], ht_ps[:, jt, :], ACTF.Gelu,
                                 bias=b1_sb[:, jt:jt + 1], scale=1.0)
        y_ps = psp.tile([OUT, B], fp32, tag="sps", bufs=2)
        for jt in range(4):
            nc.tensor.matmul(y_ps[:, :], lhsT=w2_sb[:, jt, :], rhs=g_sb[:, jt, :],
                             start=(jt == 0), stop=(jt == 3))
        y1_sb = smal.tile([OUT, B], fp32, tag="y1")
        nc.vector.tensor_scalar(y1_sb[:, :], y_ps[:, :], b2_sb[:, 0:1], 0.0,
                                ALU.add, ALU.max)
        nc.vector.tensor_scalar_min(y1_sb[:, :], y1_sb[:, :], 1.0)
        dma(bass.AP(tensor=d_y.ap().tensor, offset=0, ap=[[1, OUT], [OUT, B]]),
            y1_sb[:, :])

    nc.compile()
    return nc


def _host_constants():
    """Per-core-independent constant tensors."""
    gm = np.zeros((TL, NCH), np.float32)
    gm01 = np.zeros((TL, NCH), np.float32)
    for (c0, nch, seg) in SPLIT_TAB:
        for c in range(nch):
            gm[seg * c:seg * (c + 1), c0 + c] = 1.0 / seg
            gm01[seg * c:seg * (c + 1), c0 + c] = 1.0
    idn = np.eye(128, dtype=np.float32)
    ones = np.ones((128, 128), np.float32)
    # score-difference matrices on u_ext [130] -> s [128]
    # s_dz[t'] = u[t'+2] - u[t'+1]; s_ddz[t'] = u[t'+2] - 2u[t'+1] + u[t']
    m1 = np.zeros((TL + 2, TL), np.float32)
    m2 = np.zeros((TL + 2, TL), np.float32)
    for t in range(TL):
        m1[t + 2, t] += 1.0
        m1[t + 1, t] -= 1.0
        m2[t + 2, t] += 1.0
        m2[t + 1, t] -= 2.0
        m2[t, t] += 1.0
    return gm, gm01, idn, ones, m1, m2


def _prep_inputs(Z, q_z, q_dz, q_ddz, W1, b1, W2, b2):
    Z = np.asarray(Z, np.float32)
    W1 = np.asarray(W1, np.float32)
    gm, gm01, idn, ones, m1, m2 = _host_constants()
    q3 = np.stack([np.asarray(q_z), np.asarray(q_dz), np.asarray(q_ddz)],
                  axis=1).astype(np.float32) / math.sqrt(D)
    w2 = np.asarray(W2, np.float32).reshape(4, 128, OUT)
    b1s = np.asarray(b1, np.float32).reshape(4, 128).T.copy()
    b2s = np.asarray(b2, np.float32).reshape(OUT, 1)

    # halo-padded Z: tokens -2, -1 replicate Z[0] (makes global-boundary
    # deltas exact: dZ[0]=0, ddZ[0]=0, ddZ[1]=Z[1]-Z[0])
    Zp = np.concatenate([Z[:, 0:1], Z[:, 0:1], Z], axis=1)  # [B, T+2, D]

    in_maps = []
    for i in range(NCORES):
        t0 = TL * i
        zloc = Zp[:, t0:t0 + TL + 2]                  # [B, 130, D]
        zb = np.ascontiguousarray(zloc.transpose(1, 0, 2)).astype(BF16)
        za = np.ascontiguousarray(zloc.transpose(2, 0, 1)).astype(BF16)
        # W1 rows for this core in kernel k-tile order (sig, stat, c_local)
        rows = np.empty((KT, 128), np.int64)
        k = 0
        for sig in range(3):
            for stat in range(4):
                for (c0, nch, seg) in SPLIT_TAB:
                    for c in range(nch):
                        if seg == 16:
                            cg = 8 * i + c
                        elif seg == 32:
                            cg = 64 + 4 * i + c
                        else:
                            cg = 96 + 2 * i + c
                        base = cg * 1536 + (sig * 4 + stat) * 128
                        rows[k] = base + np.arange(128)
                        k += 1
        w1 = W1[rows.reshape(-1)].reshape(KT, 128, HIDDEN).astype(BF16)
        in_maps.append({
            "zb": zb, "za": za, "w1": w1,
            "q3": q3.astype(BF16), "gm": gm.astype(BF16),
            "gm01": gm01.astype(BF16), "idn": idn.astype(BF16),
            "ones": ones.astype(BF16), "sd1": m1[2:].copy(),
            "sd2": m2[2:].copy(), "sh1": m1[:2].copy(), "sh2": m2[:2].copy(),
            "w2": w2.astype(BF16), "b1s": b1s, "b2s": b2s,
        })
    return in_maps


def _get_nc():
    if "nc" not in _CACHE:
        _CACHE["nc"] = _build_bass()
    return _CACHE["nc"]


def _install_ntff_hook():
    import sys, types, ctypes, contextlib
    if "antenv.axon_hooks" in sys.modules:
        return
    so_path = "/opt/axon/libaxon_pjrt.so"
    if not os.path.exists(so_path):
        return
    lib = ctypes.CDLL(so_path)
    if not hasattr(lib, "axon_start_nrt_profile"):
        return
    lib.axon_start_nrt_profile.argtypes = [ctypes.POINTER(ctypes.c_int64),
                                           ctypes.c_size_t]
    lib.axon_start_nrt_profile.restype = ctypes.c_int64
    lib.axon_stop_nrt_profile.argtypes = [ctypes.c_char_p]
    lib.axon_stop_nrt_profile.restype = ctypes.c_int64

    @contextlib.contextmanager
    def _hook(output_dir, device_ids):
        import jax
        jax.devices()
        if device_ids:
            ids = (ctypes.c_int64 * len(device_ids))(*device_ids)
            rc = lib.axon_start_nrt_profile(ids, len(device_ids))
        else:
            rc = lib.axon_start_nrt_profile(None, 0)
        if rc != 0:
            raise RuntimeError(f"axon_start_nrt_profile rc={rc}")
        try:
            yield
        finally:
            n = lib.axon_stop_nrt_profile(str(output_dir).encode())
            print(f"ntff profile: {n} file(s) -> {output_dir}")

    mod = types.ModuleType("antenv.axon_hooks")
    mod.get_axon_ntff_profile_hook = lambda: _hook
    mod.set_axon_ntff_profile_hook = lambda h: None
    sys.modules["antenv.axon_hooks"] = mod


def run(trace=False, **inputs):
    from concourse.bass_utils import run_bass_kernel_spmd
    if trace:
        _install_ntff_hook()
    nc = _get_nc()
    in_maps = _prep_inputs(**inputs)
    res = run_bass_kernel_spmd(nc, in_maps, core_ids=list(range(NCORES)),
                               trace=trace)
    y = np.asarray(res.results[0]["y"], np.float32)
    return y, res


def kernel(**inputs) -> np.ndarray:
    y, _ = run(trace=False, **inputs)
    return y
